# revision 1
# baseline (speedup 1.0000x reference)
"""Trainium2 Bass kernel for a GPT-2-style transformer block.

B=4, T=1024, C=768, H=12 heads (HD=64). 8 NeuronCores.

Sharding: 2 cores per batch sequence. Each core is fed a block-permuted
copy of its sequence (own query blocks at even block positions), computes
K/V for the full sequence locally (no collectives), runs causal attention
for its 512 query tokens with data-driven masks, and the full MLP for
those tokens. Host re-assembles the [B,T,C] output.

All matmuls run in float32r (fp32 storage, TF32-like PE mode, full rate
for moving dim >= 256).
"""

import numpy as np
import ml_dtypes

P = 128
B, T, C, H = 4, 1024, 768, 12
HD = C // H        # 64
CJ = C // P        # 6 C-chunks
NT = T // P        # 8 token tiles
TQ = 512           # own query tokens per core
NQT = TQ // P      # 4 q slots
FC = 4 * C         # 3072
FCJ = FC // P      # 24
GELU_A = 0.18888357  # sqrt(0.035677408136)  -> (a*t)^2 = 0.0356774*t^2
N_CORES = 8

_CACHED = {}


def _build_nc():
    import concourse.bass as bass
    from concourse import bacc, mybir
    import concourse.tile as tile
    from concourse.masks import make_identity
    from contextlib import ExitStack

    F32 = mybir.dt.float32
    F32R = mybir.dt.float32r
    BF16 = mybir.dt.bfloat16
    AF = mybir.ActivationFunctionType
    ALU = mybir.AluOpType

    nc = bacc.Bacc()

    xp_d = nc.declare_dram_parameter("xp", [T, C], F32, isOutput=False)
    qg_d = nc.declare_dram_parameter("qg", [TQ], F32, isOutput=False)
    kg_d = nc.declare_dram_parameter("kg", [T], F32, isOutput=False)
    wqkv_d = nc.declare_dram_parameter("wqkv", [C, 3 * C], F32R, isOutput=False)
    bqkv_d = nc.declare_dram_parameter("bqkv", [3 * C], F32, isOutput=False)
    wo_d = nc.declare_dram_parameter("wo", [C, C], F32R, isOutput=False)
    bo_d = nc.declare_dram_parameter("bo", [C], F32, isOutput=False)
    wfc_d = nc.declare_dram_parameter("wfc", [C, FC], BF16, isOutput=False)
    bfc_d = nc.declare_dram_parameter("bfc", [FC], F32, isOutput=False)
    wproj_d = nc.declare_dram_parameter("wproj", [FC, C], BF16, isOutput=False)
    bproj_d = nc.declare_dram_parameter("bproj", [C], F32, isOutput=False)
    ln1s_d = nc.declare_dram_parameter("ln1s", [C], F32, isOutput=False)
    ln1b_d = nc.declare_dram_parameter("ln1b", [C], F32, isOutput=False)
    ln2s_d = nc.declare_dram_parameter("ln2s", [C], F32, isOutput=False)
    ln2b_d = nc.declare_dram_parameter("ln2b", [C], F32, isOutput=False)
    wfcns_d = nc.declare_dram_parameter("wfcns", [FC], F32, isOutput=False)
    out_d = nc.declare_dram_parameter("out", [TQ, C], F32, isOutput=True)
    import os

    def bcast_dma(engine, dst, dram_handle, offset, n):
        """DMA [n] DRAM vector broadcast across 128 partitions -> dst[128, n]."""
        ap = dram_handle[:]
        src = bass.AP(tensor=ap.tensor, offset=offset, ap=[[0, P], [1, n]])
        engine.dma_start(dst, src)

    with tile.TileContext(nc) as tc, ExitStack() as ctx:
        persist = ctx.enter_context(tc.tile_pool(name="persist", bufs=1))
        work = ctx.enter_context(tc.tile_pool(name="work", bufs=3))

        # ---------- constants / small loads ----------
        ident = persist.tile([P, P], F32, tag="ident")
        make_identity(nc, ident)
        eps_t = persist.tile([P, 1], F32, tag="eps")
        nc.vector.memset(eps_t, 1e-5)
        ones1_f32 = persist.tile([1, HD], F32, tag="ones1f")
        nc.vector.memset(ones1_f32, 1.0)
        ones1 = persist.tile([1, HD], BF16, tag="ones1")
        nc.vector.tensor_copy(ones1, ones1_f32)
        ones_col_f32 = persist.tile([P, 1], F32, tag="onescol")
        nc.vector.memset(ones_col_f32, 1.0)
        ones_col_r = persist.tile([P, 1], F32R, tag="onescolr")
        nc.vector.tensor_copy(ones_col_r, ones_col_f32)
        ones_row_bf = persist.tile([1, P], BF16, tag="onesrow")
        nc.vector.tensor_copy(ones_row_bf, ones_col_f32[0:1, 0:1].to_broadcast([1, P]))

        bqkv_po = persist.tile([P, 18], F32, tag="bqkv")
        nc.gpsimd.dma_start(bqkv_po, bqkv_d[:].rearrange("(o p) -> p o", p=P))
        bo_po = persist.tile([P, CJ], F32, tag="bo")
        nc.gpsimd.dma_start(bo_po, bo_d[:].rearrange("(o p) -> p o", p=P))
        bfc_po = persist.tile([P, FCJ], F32, tag="bfc")
        nc.gpsimd.dma_start(bfc_po, bfc_d[:].rearrange("(o p) -> p o", p=P))
        bproj_po = persist.tile([P, CJ], F32, tag="bproj")
        nc.gpsimd.dma_start(bproj_po, bproj_d[:].rearrange("(o p) -> p o", p=P))
        bv_b = persist.tile([P, C], F32, tag="bv")
        bcast_dma(nc.gpsimd, bv_b, bqkv_d, 2 * C, C)

        # ln params are folded into weights host-side; tiny loads keep the
        # dram parameters alive as kernel inputs.
        import os as _os
        if _os.environ.get("KNOSINK") != "1":
            lnsink = persist.tile([P, 4 * CJ], F32, tag="lnsink")
            for i, d in enumerate((ln1s_d, ln1b_d, ln2s_d, ln2b_d)):
                nc.gpsimd.dma_start(lnsink[:, i * CJ:(i + 1) * CJ],
                                    d[:].rearrange("(o p) -> p o", p=P))

        # masks (bf16 0/1), built later (before attention) to keep the
        # LN1 critical path clear: visible iff qg >= kg
        masks = {}
        mask_cols = {0: (0, 128), 1: (0, 128), 2: (0, 256), 3: (0, 256),
                     4: (256, 128), 5: (256, 128), 6: (256, 256), 7: (256, 256)}

        def build_masks(pool):
            kg_po = pool.tile([P, NT], F32, tag="kg")
            nc.gpsimd.dma_start(kg_po, kg_d[:].rearrange("(o p) -> p o", p=P))
            qg_b = pool.tile([P, TQ], F32, tag="qgb")
            bcast_dma(nc.gpsimd, qg_b, qg_d, 0, TQ)
            for kc in range(NT):
                off, w = mask_cols[kc]
                m = persist.tile([P, w], BF16, tag=f"mask{kc}", name=f"mask{kc}")
                nc.vector.tensor_scalar(
                    m, qg_b[:, off:off + w], kg_po[:, kc:kc + 1], None, ALU.is_ge)
                masks[kc] = m

        # own x tiles (even permuted positions) persist until phase 4
        x_own = [persist.tile([P, C], F32, tag=f"xo{t}", name=f"xo{t}")
                 for t in range(NQT)]
        for t in range(NQT):
            nc.sync.dma_start(x_own[t], xp_d[2 * t * P:(2 * t + 1) * P, :])
        x1T = [persist.tile([P, TQ], F32R, tag=f"x1T{m}", name=f"x1T{m}")
               for m in range(CJ)]

        with tc.tile_pool(name="attn_live", bufs=1) as attn_live:
            kT = [attn_live.tile([P, T], BF16, tag=f"kT{h}", name=f"kT{h}")
                  for h in range(H)]
            qT = [attn_live.tile([P, TQ], BF16, tag=f"qT{h}", name=f"qT{h}")
                  for h in range(H)]
            for h in range(H):
                zlo = slice(0, HD) if h % 2 else slice(HD, P)
                nc.vector.memset(kT[h][zlo, :], 0.0)
                nc.vector.memset(qT[h][zlo, :], 0.0)
            v_aug = [attn_live.tile([P, H, HD + 1], BF16, tag=f"vaug{t}",
                                    name=f"vaug{t}") for t in range(NT)]
            yT = [attn_live.tile([P, TQ], F32R, tag=f"yT{j}", name=f"yT{j}")
                  for j in range(CJ)]

            # ---- phases 1-3 fused: LN1+transpose, QKV, attention ----
            with tc.tile_pool(name="ph12", bufs=1) as ph12, \
                 tc.tile_pool(name="wstream", bufs=1) as wstream, \
                 tc.tile_pool(name="att", bufs=3) as att, \
                 tc.tile_pool(name="maskb_p", bufs=1) as maskb_p, \
                 tc.tile_pool(name="ps_tr1", bufs=2, space="PSUM") as ps_tr, \
                 tc.tile_pool(name="ps_mm", bufs=3, space="PSUM") as ps_mm, \
                 tc.tile_pool(name="ps_av", bufs=2, space="PSUM") as ps_av, \
                 tc.tile_pool(name="ps_bc", bufs=1, space="PSUM") as ps_bc:
                build_masks(maskb_p)

                x_sb = []
                for t in range(NT):
                    if t % 2 == 0:
                        x_sb.append(x_own[t // 2])
                    else:
                        xt = ph12.tile([P, C], F32, tag=f"x{t}", name=f"x{t}")
                        nc.sync.dma_start(xt, xp_d[t * P:(t + 1) * P, :])
                        x_sb.append(xt)

                xlnT = [ph12.tile([P, T], F32R, tag=f"xlnT{j}", name=f"xlnT{j}")
                        for j in range(CJ)]

                for t in range(NT):
                    stats = work.tile([P, 3, 6], F32, tag="bnstats")
                    for g in range(3):
                        nc.vector.bn_stats(stats[:, g, :],
                                           x_sb[t][:, g * 256:(g + 1) * 256])
                    mv = work.tile([P, 2], F32, tag="bnmv")
                    nc.vector.bn_aggr(mv, stats)
                    rstd = work.tile([P, 1], F32, tag="rstd")
                    nc.scalar.activation(rstd, mv[:, 1:2], AF.Sqrt, bias=eps_t)
                    nc.vector.reciprocal(rstd, rstd)
                    nmr = work.tile([P, 1], F32, tag="nmr")
                    nc.vector.tensor_tensor(nmr, mv[:, 0:1], rstd, ALU.mult)
                    nc.vector.tensor_scalar(nmr, nmr, -1.0, None, ALU.mult)
                    xln = work.tile([P, C], F32, tag="xln")
                    nc.scalar.activation(xln, x_sb[t], AF.Identity,
                                         bias=nmr, scale=rstd)
                    for j in range(CJ):
                        ptr = ps_tr.tile([P, P], F32, tag="tr")
                        nc.tensor.transpose(ptr, xln[:, j * P:(j + 1) * P], ident)
                        dst = xlnT[j][:, t * P:(t + 1) * P]
                        if j % 2 == 0:
                            nc.vector.tensor_copy(dst, ptr)
                        else:
                            nc.scalar.copy(dst, ptr)

                wq3 = wqkv_d[:, :].rearrange("(o p) n -> p o n", p=P)

                # v (token-major) + ones column -> v_aug (2 Wv pieces)
                for t in range(NT):
                    nc.vector.tensor_copy(
                        v_aug[t][:, :, HD:HD + 1],
                        ones_col_f32.to_broadcast([P, H, 1]))
                for half in range(2):
                    wt = wstream.tile([P, CJ, 384], F32R, tag="wpiece")
                    nc.gpsimd.dma_start(
                        wt, wq3[:, :, 2 * C + half * 384: 2 * C + (half + 1) * 384])
                    for t in range(NT):
                        pm = ps_mm.tile([P, 512], F32, tag="mm")
                        pmv = pm[:, 0:384]
                        for kc in range(CJ):
                            nc.tensor.matmul(
                                pmv, xlnT[kc][:, t * P:(t + 1) * P],
                                wt[:, kc, :],
                                start=(kc == 0), stop=(kc == CJ - 1))
                        nc.vector.tensor_tensor(
                            v_aug[t][:, half * 6:(half + 1) * 6, 0:HD],
                            pmv.rearrange("p (h d) -> p h d", d=HD),
                            bv_b[:, half * 384:(half + 1) * 384].rearrange(
                                "p (h d) -> p h d", d=HD),
                            ALU.add)

                # per weight piece: kT[j], qT[j], then attention for tile j
                for kp in range(2):
                    wtk = wstream.tile([P, CJ, 384], F32R, tag="wpiecek",
                                       name="wtk")
                    nc.gpsimd.dma_start(
                        wtk, wq3[:, :, C + kp * 384: C + (kp + 1) * 384])
                    wtq = wstream.tile([P, CJ, 384], F32R, tag="wpieceq",
                                       name="wtq")
                    nc.gpsimd.dma_start(
                        wtq, wq3[:, :, kp * 384:(kp + 1) * 384])
                    for jl in range(3):
                        j = 3 * kp + jl
                        for half in range(2):
                            pm = ps_mm.tile([P, 512], F32, tag="mm")
                            for kc in range(CJ):
                                nc.tensor.matmul(
                                    pm, wtk[:, kc, jl * P:(jl + 1) * P],
                                    xlnT[kc][:, half * 512:(half + 1) * 512],
                                    start=(kc == 0), stop=(kc == CJ - 1))
                            for hh in range(2):
                                hs = slice(hh * HD, (hh + 1) * HD)
                                nc.vector.tensor_scalar(
                                    kT[2 * j + hh][hs,
                                                   half * 512:(half + 1) * 512],
                                    pm[hs, :],
                                    bqkv_po[hs, CJ + j:CJ + j + 1], None,
                                    ALU.add)
                        pm = ps_mm.tile([P, 512], F32, tag="mm")
                        for kc in range(CJ):
                            own = xlnT[kc].rearrange(
                                "p (b c) -> p b c", c=P)[:, 0::2, :]
                            nc.tensor.matmul(
                                pm, wtq[:, kc, jl * P:(jl + 1) * P], own,
                                start=(kc == 0), stop=(kc == CJ - 1))
                        for hh in range(2):
                            hs = slice(hh * HD, (hh + 1) * HD)
                            nc.vector.tensor_scalar(
                                qT[2 * j + hh][hs, :], pm[hs, :],
                                bqkv_po[hs, j:j + 1], None, ALU.add)

                        # ---- attention for the two heads of tile j ----
                        for hh in range(2):
                            h = 2 * j + hh
                            base = hh * HD
                            av = ps_av.tile([HD + 1, 512], F32, tag="av")
                            for kc in range(NT):
                                n0 = 0 if kc < 4 else 256
                                w = 512 - n0
                                sc = ps_mm.tile([P, 512], F32, tag="mm")
                                nc.tensor.matmul(
                                    sc[:, 0:w],
                                    kT[h][:, kc * P:(kc + 1) * P],
                                    qT[h][:, n0:512],
                                    start=True, stop=True)
                                ex = att.tile([P, 512], BF16, tag="exp")
                                nc.scalar.activation(ex[:, 0:w], sc[:, 0:w],
                                                     AF.Exp, scale=0.125)
                                off, wm = mask_cols[kc]
                                loc = off - n0
                                nc.vector.tensor_tensor(
                                    ex[:, loc:loc + wm], ex[:, loc:loc + wm],
                                    masks[kc], ALU.mult)
                                nc.tensor.matmul(
                                    av[:, n0:512], v_aug[kc][:, h, :],
                                    ex[:, 0:w],
                                    start=(kc == 0), stop=(kc == NT - 1),
                                    skip_group_check=True)
                            sums_bf = att.tile([1, 512], BF16, tag="sums")
                            nc.vector.tensor_copy(sums_bf, av[HD:HD + 1, :])
                            bc = ps_bc.tile([HD, 512], F32, tag="bc")
                            nc.tensor.matmul(bc, ones1, sums_bf,
                                             start=True, stop=True)
                            rb = att.tile([HD, 512], F32, tag="rb")
                            with nc.allow_low_precision(reason="softmax denom"):
                                nc.vector.reciprocal_approx_fast(rb, bc)
                            nc.vector.tensor_tensor(
                                yT[j][base:base + HD, :], av[0:HD, :], rb,
                                ALU.mult)

            # ---- phase 4: x_own^T (transpose-accumulate) + Wo -> x1T ----
            with tc.tile_pool(name="wo_p", bufs=1) as wo_p:
                wo_t = wo_p.tile([P, CJ, C], F32R, tag="wo")
                nc.gpsimd.dma_start(
                    wo_t, wo_d[:, :].rearrange("(o p) n -> p o n", p=P))
                with tc.tile_pool(name="ps_mm4", bufs=2, space="PSUM") as ps_mm4:
                    for m in range(CJ):
                        pm = ps_mm4.tile([P, TQ], F32, tag="mm")
                        for t in range(NQT):
                            nc.tensor.matmul(
                                pm[:, t * P:(t + 1) * P],
                                x_own[t][:, m * P:(m + 1) * P], ident,
                                is_transpose=True,
                                start=(t == 0), stop=False,
                                skip_group_check=True)
                        for kc in range(CJ):
                            nc.tensor.matmul(
                                pm, wo_t[:, kc, m * P:(m + 1) * P], yT[kc],
                                start=False, stop=(kc == CJ - 1),
                                skip_group_check=True)
                        with nc.allow_low_precision(reason="residual f32r"):
                            nc.vector.tensor_scalar(
                                x1T[m], pm, bo_po[:, m:m + 1], None, ALU.add)

        # ---------- phases 5-7: LN2 (feature-major), FC+gelu, proj+out ----------
        with tc.tile_pool(name="mlp_live", bufs=1) as mlp_live, \
             tc.tile_pool(name="ln2c_p", bufs=1) as ln2c_p:
            h1T = [mlp_live.tile([P, TQ], BF16, tag=f"h1T{m}", name=f"h1T{m}")
                   for m in range(FCJ)]

            with tc.tile_pool(name="ph5", bufs=3) as ph5, \
                 tc.tile_pool(name="ps_st", bufs=1, space="PSUM") as ps_st, \
                 tc.tile_pool(name="ps_bc2", bufs=1, space="PSUM") as ps_bc2:
                # token stats via ones-column matmuls (partition reduction)
                mu_ps = ps_st.tile([1, TQ], F32, tag="mups", name="mups")
                sq_ps = ps_st.tile([1, TQ], F32, tag="sqps", name="sqps")
                for m in range(CJ):
                    nc.tensor.matmul(mu_ps, ones_col_r, x1T[m],
                                     start=(m == 0), stop=(m == CJ - 1))
                for m in range(CJ):
                    sq = ph5.tile([P, TQ], F32R, tag="sq")
                    if m % 2 == 0:
                        nc.scalar.activation(sq, x1T[m], AF.Square)
                    else:
                        nc.vector.tensor_tensor(sq, x1T[m], x1T[m], ALU.mult)
                    nc.tensor.matmul(sq_ps, ones_col_r, sq,
                                     start=(m == 0), stop=(m == CJ - 1))
                mu_f = ln2c_p.tile([1, TQ], F32, tag="muf")
                nc.vector.tensor_scalar(mu_f, mu_ps, 1.0 / C, None, ALU.mult)
                mu_bf = ln2c_p.tile([1, TQ], BF16, tag="mubf")
                nc.vector.tensor_copy(mu_bf, mu_f)
                var_f = ln2c_p.tile([1, TQ], F32, tag="varf")
                nc.vector.tensor_scalar(var_f, sq_ps, 1.0 / C, None, ALU.mult)
                musq = ln2c_p.tile([1, TQ], F32, tag="musq")
                nc.vector.tensor_tensor(musq, mu_f, mu_f, ALU.mult)
                nc.vector.tensor_tensor(var_f, var_f, musq, ALU.subtract)
                sd_f = ln2c_p.tile([1, TQ], F32, tag="sdf")
                nc.scalar.activation(sd_f, var_f, AF.Sqrt, bias=eps_t[0:1, :])
                sd_bf = ln2c_p.tile([1, TQ], BF16, tag="sdbf")
                nc.vector.tensor_copy(sd_bf, sd_f)
                rstd_f = ln2c_p.tile([1, TQ], F32, tag="rstdf")
                nc.vector.reciprocal_approx_fast(rstd_f, sd_f)
                rstd_bf = ln2c_p.tile([1, TQ], BF16, tag="rstdbf")
                nc.vector.tensor_copy(rstd_bf, rstd_f)
                # broadcast rstd across partitions via K=1 matmul
                rstd_bc_ps = ps_bc2.tile([P, TQ], F32, tag="rstdbc",
                                         name="rstdbc")
                nc.tensor.matmul(rstd_bc_ps, ones_row_bf, rstd_bf,
                                 start=True, stop=True)
                rstd_bc = ln2c_p.tile([P, TQ], F32, tag="rstdbcs")
                nc.vector.tensor_copy(rstd_bc, rstd_bc_ps)
                # bf16 copy of x1T for the bf16 FC matmul
                x1Tbf = [ln2c_p.tile([P, TQ], BF16, tag=f"x1Tbf{m}",
                                     name=f"x1Tbf{m}") for m in range(CJ)]
                for m in range(CJ):
                    nc.vector.tensor_copy(x1Tbf[m], x1T[m])
                # negated Wfc column sums (rank-1 mean correction)
                wfcns_bf = ln2c_p.tile([1, FC], BF16, tag="wfcns")
                nc.gpsimd.dma_start(
                    wfcns_bf, wfcns_d[:].rearrange("(a c) -> a c", a=1))
                bfc_bf = ln2c_p.tile([1, FC], BF16, tag="bfcbf")
                nc.gpsimd.dma_start(
                    bfc_bf, bfc_d[:].rearrange("(a c) -> a c", a=1))

            # ---------- phase 6: FC + gelu (LN2 folded in) ----------
            with tc.tile_pool(name="wfc_p", bufs=3) as wfc_p, \
                 tc.tile_pool(name="ph6", bufs=3) as ph6, \
                 tc.tile_pool(name="ps_mm6", bufs=4, space="PSUM") as ps_mm6:
                wfc3 = wfc_d[:, :].rearrange("(o p) n -> p o n", p=P)
                for m in range(FCJ):
                    if m % 4 == 0:
                        wt4 = wfc_p.tile([P, CJ, 512], BF16, tag="wfc")
                        nc.sync.dma_start(
                            wt4, wfc3[:, :, m * P:(m + 4) * P])
                    ml = m % 4
                    pm = ps_mm6.tile([P, TQ], F32, tag="mm")
                    for kc in range(CJ):
                        nc.tensor.matmul(pm, wt4[:, kc, ml * P:(ml + 1) * P],
                                         x1Tbf[kc],
                                         start=(kc == 0), stop=False,
                                         skip_group_check=True)
                    # rank-1: -colsum (x) mu  and  bfc (x) sd
                    nc.tensor.matmul(pm, wfcns_bf[:, m * P:(m + 1) * P],
                                     mu_bf, start=False, stop=False,
                                     skip_group_check=True)
                    nc.tensor.matmul(pm, bfc_bf[:, m * P:(m + 1) * P],
                                     sd_bf, start=False, stop=True,
                                     skip_group_check=True)
                    xb = ph6.tile([P, TQ], F32, tag="xb")
                    nc.vector.tensor_tensor(xb, pm, rstd_bc, ALU.mult)
                    tsq = ph6.tile([P, TQ], F32, tag="tsq")
                    nc.scalar.activation(tsq, xb, AF.Square)
                    nc.scalar.activation(tsq, tsq, AF.Square, scale=GELU_A)
                    u = ph6.tile([P, TQ], F32, tag="u")
                    nc.scalar.activation(u, tsq, AF.Tanh)
                    nc.vector.tensor_scalar(u, u, 0.5, 0.5, ALU.mult, ALU.add)
                    nc.vector.tensor_tensor(h1T[m], xb, u, ALU.mult)

            # ---------- phase 7: proj + residual -> out ----------
            with tc.tile_pool(name="wpj_p", bufs=4) as wpj_p, \
                 tc.tile_pool(name="ph7", bufs=2) as ph7, \
                 tc.tile_pool(name="out_p", bufs=1) as out_p, \
                 tc.tile_pool(name="ps_tr7", bufs=2, space="PSUM") as ps_tr, \
                 tc.tile_pool(name="ps_pj", bufs=1, space="PSUM") as ps_pj:
                pms = [ps_pj.tile([P, TQ], F32, tag=f"pj{m}", name=f"pj{m}")
                       for m in range(CJ)]
                wpj3 = wproj_d[:, :].rearrange("(o p) n -> p o n", p=P)
                for kc in range(FCJ):
                    if kc % 2 == 0:
                        wt2 = wpj_p.tile([P, 2, C], BF16, tag="wpj")
                        nc.sync.dma_start(
                            wt2, wpj3[:, kc:kc + 2, :])
                    kl = kc % 2
                    for m in range(CJ):
                        nc.tensor.matmul(
                            pms[m], wt2[:, kl, m * P:(m + 1) * P], h1T[kc],
                            start=(kc == 0), stop=(kc == FCJ - 1))
                out_sb = [out_p.tile([P, C], F32, tag=f"osb{t}", name=f"osb{t}")
                          for t in range(NQT)]
                for m in range(CJ):
                    ojT = ph7.tile([P, TQ], F32, tag="ojT")
                    nc.vector.tensor_scalar(
                        ojT, pms[m], bproj_po[:, m:m + 1], None, ALU.add)
                    nc.vector.tensor_tensor(ojT, ojT, x1T[m], ALU.add)
                    for t in range(NQT):
                        ptr = ps_tr.tile([P, P], F32, tag="tr")
                        nc.tensor.transpose(
                            ptr, ojT[:, t * P:(t + 1) * P], ident)
                        dst = out_sb[t][:, m * P:(m + 1) * P]
                        if m % 2 == 0:
                            nc.vector.tensor_copy(dst, ptr)
                        else:
                            nc.scalar.copy(dst, ptr)
                for t in range(NQT):
                    nc.sync.dma_start(out_d[t * P:(t + 1) * P, :], out_sb[t])

    nc.compile()
    return nc


def _get_nc():
    if "nc" not in _CACHED:
        _CACHED["nc"] = _build_nc()
    return _CACHED["nc"]


def _perm_blocks(p):
    return [p, 1 - p, 2 + p, 3 - p, 4 + p, 5 - p, 6 + p, 7 - p]


def kernel(x, ln1_scale, ln1_bias, Wqkv, bqkv, Wo, bo,
           ln2_scale, ln2_bias, Wfc, bfc, Wproj, bproj):
    from concourse.bass_utils import run_bass_kernel_spmd

    x = np.asarray(x, np.float32)
    # Fold LN scale/bias into the following projection (exact):
    #   ln(x)*s + b  @ W  ==  ln(x) @ (s[:,None]*W)  +  b @ W
    Wqkv64 = np.asarray(Wqkv, np.float64)
    Wqkv64 = np.asarray(ln1_scale, np.float64)[:, None] * Wqkv64
    bqkv64 = np.asarray(bqkv, np.float64) + np.asarray(ln1_bias, np.float64) @ Wqkv64
    Wfc64 = np.asarray(Wfc, np.float64)
    Wfc64 = np.asarray(ln2_scale, np.float64)[:, None] * Wfc64
    bfc64 = np.asarray(bfc, np.float64) + np.asarray(ln2_bias, np.float64) @ Wfc64
    Wfc = Wfc64.astype(np.float32)
    bfc = bfc64.astype(np.float32)
    wfcns = (-Wfc64.sum(axis=0)).astype(np.float32)
    # Reference splits qkv per head: columns are [h0: q|k|v, h1: q|k|v, ...].
    # Permute to the kernel's [Q(768) | K(768) | V(768)] layout.
    colmap = np.arange(3 * C).reshape(H, 3, HD)
    qkv_perm = np.concatenate(
        [colmap[:, 0, :].ravel(), colmap[:, 1, :].ravel(), colmap[:, 2, :].ravel()])
    Wqkv = Wqkv64.astype(np.float32)[:, qkv_perm]
    bqkv = bqkv64.astype(np.float32)[qkv_perm]
    shared = {
        "wqkv": np.ascontiguousarray(Wqkv),
        "bqkv": np.ascontiguousarray(bqkv),
        "wo": np.asarray(Wo, np.float32),
        "bo": np.asarray(bo, np.float32),
        "wfc": np.ascontiguousarray(Wfc.astype(ml_dtypes.bfloat16)),
        "bfc": np.ascontiguousarray(bfc),
        "wfcns": np.ascontiguousarray(wfcns),
        "wproj": np.ascontiguousarray(np.asarray(Wproj, np.float32).astype(ml_dtypes.bfloat16)),
        "bproj": np.asarray(bproj, np.float32),
        "ln1s": np.asarray(ln1_scale, np.float32),
        "ln1b": np.asarray(ln1_bias, np.float32),
        "ln2s": np.asarray(ln2_scale, np.float32),
        "ln2b": np.asarray(ln2_bias, np.float32),
    }
    in_maps = []
    own_toks = []
    for c in range(N_CORES):
        s, p = divmod(c, 2)
        blocks = _perm_blocks(p)
        tok = np.concatenate([np.arange(b * P, (b + 1) * P) for b in blocks])
        own = np.concatenate([np.arange(b * P, (b + 1) * P) for b in blocks[0::2]])
        own_toks.append((s, own))
        in_maps.append({
            "xp": np.ascontiguousarray(x[s][tok]),
            "qg": own.astype(np.float32),
            "kg": tok.astype(np.float32),
            **shared,
        })

    nc = _get_nc()
    res = run_bass_kernel_spmd(nc, in_maps, list(range(N_CORES)))

    out = np.empty((B, T, C), np.float32)
    for c in range(N_CORES):
        s, own = own_toks[c]
        out[s][own] = res.results[c]["out"]
    return out

